# revision 1
# baseline (speedup 1.0000x reference)
"""Trainium2 Bass kernel for nn_CWT_54872502174093.

CWT of a batch of signals with a 64-scale mexican-hat filter bank.

Math: the reference computes, per scale s (1..64),
    coef[b,s,n] = -sqrt(s) * diff(conv_full(x[b], K_s))[start_s + n]
which is algebraically a direct correlation
    coef[b,s,n] = sum_k g_s[k] * x[b, n + k - SH_s]
with g_s = reversed(-sqrt(s) * diff-extended(K_s)) (16s+2 taps) and
SH_s = len(K_s) - start_s - 1 = 8s+1.

On-chip this is evaluated as banded-Toeplitz matmuls in float32r
(1 cycle/row for moving free >= 256 on TRN2):
  stationary = transposed signal chunk [128 pos, (2 n-halves x 64 batch)]
  moving     = host-precomputed skewed filter matrix slice [128, 256]
  PSUM accumulates over tap chunks m.
Sharding: data-parallel over batch, 64 batches per core, 8 cores.
"""

import numpy as np

SIG = 2048
NS = 64
NCORES = 8
BC = 64          # batch per core
NCHUNK = 26      # position chunks of 128 in the padded transposed signal
LPAD = 5         # left zero-pad chunks (640 positions)
J = 256          # output tile width (moving free size)
NQ = 4           # q groups: n0 = 256*q
NH = 2           # n halves packed into stationary columns (offset 1024)
WMAX = 1536      # max filter-matrix width (scale 64)

_CACHE = {}


def _build_filters(kernels, scales_sqrt, trim_idx):
    """Per-scale skewed filter matrices W_s[p, i] = g_s[i0 + p - i], packed
    side by side into one [128, totcols] f32 array. Returns (Wall, specs)
    with specs[s-1] = (coloff, m_hi, c)."""
    kernels = np.asarray(kernels, np.float64)
    scales_sqrt = np.asarray(scales_sqrt, np.float64)
    trim_idx = np.asarray(trim_idx)
    cols = []
    specs = []
    totcols = 0
    for s in range(1, NS + 1):
        L = 16 * s + 1
        Karr = np.zeros(L + 2, np.float64)
        Karr[1:L + 1] = kernels[s - 1, :L]
        Wl = -scales_sqrt[s - 1] * (Karr[1:] - Karr[:-1])  # len L+1
        g = Wl[::-1].copy()
        Ks = L + 1
        start = int(trim_idx[s - 1, 0])
        SH = L - start - 1
        c = -(-SH // 128)
        m_hi = (Ks + 254 - SH) // 128 + c
        i0 = 128 * (m_hi - c) + SH
        Wd = 128 * m_hi + J
        P = np.arange(128)[:, None]
        I = np.arange(Wd)[None, :]
        gi = i0 + P - I
        Wbuf = np.where((gi >= 0) & (gi < Ks), g[np.clip(gi, 0, Ks - 1)], 0.0)
        cols.append(Wbuf.astype(np.float32))
        specs.append((totcols, m_hi, c))
        totcols += Wd
    return np.ascontiguousarray(np.concatenate(cols, axis=1)), specs


def _build_nc(totcols, specs, repeat=1, variant="full"):
    """variant: timing ablations — "full" (real kernel), "now" (skip W DMAs,
    reuse one stale tile: wrong numerics), "noout" (skip output DMAs),
    "nomm" (skip matmuls+copies: DMA streams only)."""
    import concourse.bacc as bacc
    import concourse.mybir as mybir
    import concourse.tile as tile

    f32 = mybir.dt.float32
    f32r = mybir.dt.float32r
    nc = bacc.Bacc(None, target_bir_lowering=False)
    # xt free layout: (chunk c, h*64+b) with the h=1 column block holding
    # chunk c+8 — the two packed n-halves must be contiguous because the
    # matmul stationary AP only allows one free dimension.
    xt_d = nc.declare_dram_parameter("xt", [128, NCHUNK, NH * BC], f32r,
                                     isOutput=False)
    w_d = nc.declare_dram_parameter("w", [128, totcols], f32r, isOutput=False)
    out_d = nc.declare_dram_parameter("out", [BC, NS, SIG], f32, isOutput=True)
    out_v = out_d.ap().rearrange("b s (h q j) -> s h b q j", h=NH, q=NQ, j=J)

    with tile.TileContext(nc) as tc:
        with tc.tile_pool(name="xtp", bufs=1) as xtp, \
             tc.tile_pool(name="wp", bufs=3) as wp, \
             tc.tile_pool(name="pp", bufs=8, space="PSUM") as pp, \
             tc.tile_pool(name="sp", bufs=3) as sp:
            xt = xtp.tile([128, NCHUNK, NH * BC], f32r)
            nc.sync.dma_start(xt[:], xt_d.ap())
            xt_r = xt[:]
            stale = None
            for s in [s for _ in range(repeat) for s in range(1, NS + 1)]:
                coloff, m_hi, c = specs[s - 1]
                Wd = 128 * m_hi + J
                if variant == "now" and stale is not None:
                    wt = stale
                else:
                    wt = wp.tile([128, WMAX], f32r, tag="w")
                    nc.sync.dma_start(wt[:, :Wd] if variant != "now" else wt[:],
                                      w_d.ap()[:, coloff:coloff + Wd]
                                      if variant != "now" else w_d.ap()[:, :WMAX])
                    stale = wt
                wt_r = wt[:]
                stage = sp.tile([128, NQ, J], f32, tag="stage")
                if variant != "nomm":
                    for q in range(NQ):
                        ps = pp.tile([128, J], f32)
                        for m in range(m_hi + 1):
                            ci = LPAD + 2 * q - c + m
                            stat = xt_r[:, ci, :]
                            mov = wt_r[:, 128 * (m_hi - m):128 * (m_hi - m) + J]
                            nc.tensor.matmul(ps[:], stat, mov,
                                             start=(m == 0), stop=(m == m_hi))
                        nc.vector.tensor_copy(stage[:, q, :], ps[:])
                else:
                    nc.vector.tensor_copy(stage[:, 0, 0:1], xt[:, 0, 0:1])
                if variant != "noout":
                    for h in range(NH):
                        nc.scalar.dma_start(out_v[s - 1, h],
                                            stage[64 * h:64 * h + 64, :, :])
    nc.compile()
    return nc


def _build_filters_b(kernels, scales_sqrt, trim_idx):
    """Plan B: per-scale Toeplitz stationary tiles [128, 128] with J=128,
    packed side by side. Per scale, the signal-window phase phi in {0, 64}
    is chosen to minimize the tile count (reaches the 360-tile floor).
    specs[s-1] = (coloff, m_hi, c, phi)."""
    kernels = np.asarray(kernels, np.float64)
    scales_sqrt = np.asarray(scales_sqrt, np.float64)
    trim_idx = np.asarray(trim_idx)
    JB = 128
    cols = []
    specs = []
    totcols = 0
    for s in range(1, NS + 1):
        L = 16 * s + 1
        Karr = np.zeros(L + 2, np.float64)
        Karr[1:L + 1] = kernels[s - 1, :L]
        Wl = -scales_sqrt[s - 1] * (Karr[1:] - Karr[:-1])
        g = Wl[::-1].copy()
        Ks = L + 1
        start = int(trim_idx[s - 1, 0])
        SH = L - start - 1

        def _cnt(phi):
            Sp = SH + phi
            c = -(-Sp // 128)
            return (Ks + JB - 2 - Sp) // 128 + c + 1

        phi = min((0, 64), key=_cnt)
        Sp = SH + phi
        c = -(-Sp // 128)
        m_hi = (Ks + JB - 2 - Sp) // 128 + c
        # tile m: T[p, j] = g[u_m + p - j], u_m = 128*(m-c) + SH + phi
        P = np.arange(128)[:, None]
        Jv = np.arange(JB)[None, :]
        for m in range(m_hi + 1):
            u = 128 * (m - c) + Sp
            gi = u + P - Jv
            T = np.where((gi >= 0) & (gi < Ks), g[np.clip(gi, 0, Ks - 1)], 0.0)
            cols.append(T.astype(np.float32))
        specs.append((totcols, m_hi, c, phi))
        totcols += (m_hi + 1) * JB
    return np.ascontiguousarray(np.concatenate(cols, axis=1)), specs


def _build_nc_b(totcols, specs, repeat=1, wdtype="f16", variant="full"):
    """Plan B: stationary = Toeplitz filter tile [128 taps, 128 j], moving =
    signal columns [128 taps, 8 n-spread x 64 batch], PSUM j-major.
    Output DRAM layout [s, j, g, k, b] fp16; host upcasts + transposes.
    v2: bf16 operands (no DVE upcast), fp16 output staging, evacuation
    split across DVE and ACT, output DMAs batched OBATCH scales at a time.
    variant: timing ablations — "full" | "now" (stale W, wrong numerics) |
    "noout" (skip out DMAs) | "nomm" (skip matmuls+copies) |
    "mmonly" (stale W + matmuls only) | "mmevac" (stale W + matmuls + evac)."""
    import concourse.bacc as bacc
    import concourse.mybir as mybir
    import concourse.tile as tile

    f32 = mybir.dt.float32
    f32r = mybir.dt.float32r
    f16 = mybir.dt.float16
    bf16 = mybir.dt.bfloat16
    # wdtype: "bf16" = both operands bf16 (1 col/cyc — slow)
    #         "f32r_up" = xt f32r, W fp16 upcast to f32r on ACT
    #         "f32r_wf16" = xt f32r moving, W fp16 stationary direct
    mov_dt = bf16 if wdtype == "bf16" else f32r
    wst_dt = {"bf16": bf16, "f32r_up": f32r, "f32r_wf16": f16}[wdtype]
    wdr_dt = {"bf16": bf16, "f32r_up": f16, "f32r_wf16": f16}[wdtype]
    NCC = 11   # xt3 chunk-offset axis (cc = g - c + m + CPADB in [0, 10])
    nc = bacc.Bacc(None, target_bir_lowering=False)
    xt_d = nc.declare_dram_parameter("xt", [128, 2, NCC, 512], mov_dt,
                                     isOutput=False)
    w_d = nc.declare_dram_parameter("w", [128, totcols], wdr_dt, isOutput=False)
    out_d = nc.declare_dram_parameter(
        "out", [NS // OBATCH, 128, OBATCH, 2, 8, BC], f16, isOutput=True)
    out_v = out_d.ap()

    # W group-DMA plan: GRP scales per DMA (few large transfers hide W
    # latency under PE work). Group g covers scales [GRP*g+1, GRP*(g+1)].
    GRP = 16
    NGRP = NS // GRP
    goff = []
    gcols = []
    for g in range(NGRP):
        c0 = specs[GRP * g][0]
        ce = specs[GRP * (g + 1) - 1]
        goff.append(c0)
        gcols.append(ce[0] + (ce[1] + 1) * 128 - c0)
    GWMAX = max(gcols)

    with tile.TileContext(nc) as tc:
        with tc.tile_pool(name="xtp", bufs=1) as xtp, \
             tc.tile_pool(name="wp", bufs=2) as wp, \
             tc.tile_pool(name="wp16", bufs=2) as wp16, \
             tc.tile_pool(name="pp",
                          bufs=4 if (variant in ("mmint", "intonly")
                                     or INTERLEAVE) else 8,
                          space="PSUM") as pp, \
             tc.tile_pool(name="sp", bufs=3) as sp:
            xt = xtp.tile([128, 2, NCC, 512], mov_dt)
            nc.sync.dma_start(xt[:], xt_d.ap())
            slist = [s for _ in range(repeat) for s in range(1, NS + 1)]
            wts = {}
            stale = None

            def fetch_group(gi):
                """Issue the W group DMA for group index gi (mod NGRP)."""
                nonlocal stale
                if gi >= len(slist) // GRP:
                    return
                g = gi % NGRP
                if variant in ("now", "mmonly", "mmevac", "mm32", "mmn256", "mmnoacc", "mmint", "intonly") and stale is not None:
                    wts[gi] = stale
                    return
                wt = wp.tile([128, GWMAX], wdr_dt, tag="w")
                nc.sync.dma_start(wt[:, :gcols[g]],
                                  w_d.ap()[:, goff[g]:goff[g] + gcols[g]])
                if wdtype == "f32r_up":
                    wtr = wp16.tile([128, GWMAX], f32r, tag="wr")
                    nc.vector.tensor_copy(wtr[:, :gcols[g]], wt[:, :gcols[g]])
                    wt = wtr
                wts[gi] = wt
                stale = wt

            stage = None
            fetch_group(0)
            fetch_group(1)
            for idx, s in enumerate(slist):
                coloff, m_hi, c, phi = specs[s - 1]
                gi = idx // GRP
                if idx % GRP == 0:
                    fetch_group(gi + 2)
                    wts.pop(gi - 2, None)
                wt_g = wts[gi]
                loff = coloff - goff[gi % NGRP]
                sb = idx % OBATCH
                if sb == 0:
                    stage = sp.tile([128, OBATCH, 2, 512], f16, tag="stage")
                if variant == "mm32":
                    # timing probe: 32-col stationaries (wrong numerics)
                    for g in range(2):
                        ps = pp.tile([32, 512], f32)
                        for m in range(m_hi + 1):
                            cc = g - c + m + CPADB
                            wlo = loff + 128 * m
                            nc.tensor.matmul(ps[:],
                                             wt_g[:, wlo:wlo + 32],
                                             xt[:, phi // 64, cc, :],
                                             start=(m == 0), stop=(m == m_hi))
                elif variant == "mmn256":
                    # timing probe: N=256 moving (wrong numerics)
                    for g in range(2):
                        ps = pp.tile([128, 256], f32)
                        for m in range(m_hi + 1):
                            cc = g - c + m + CPADB
                            wlo = loff + 128 * m
                            nc.tensor.matmul(ps[:],
                                             wt_g[:, wlo:wlo + 128],
                                             xt[:, phi // 64, cc, 0:256],
                                             start=(m == 0), stop=(m == m_hi))
                elif variant == "mmnoacc":
                    # timing probe: no accumulation, every MM start+stop
                    for g in range(2):
                        ps = pp.tile([128, 512], f32)
                        for m in range(m_hi + 1):
                            cc = g - c + m + CPADB
                            wlo = loff + 128 * m
                            nc.tensor.matmul(ps[:],
                                             wt_g[:, wlo:wlo + 128],
                                             xt[:, phi // 64, cc, :],
                                             start=True, stop=True)
                elif variant in ("mmint", "intonly") or (
                        variant in ("full", "now", "noout") and INTERLEAVE):
                    # interleave the two g-chains: adjacent PE instructions
                    # target different PSUM banks so fill/drain can overlap
                    ps0 = pp.tile([128, 512], f32)
                    ps1 = pp.tile([128, 512], f32)
                    pss = (ps0, ps1)
                    for m in range(m_hi + 1):
                        for g in range(2):
                            cc = g - c + m + CPADB
                            wlo = loff + 128 * m
                            nc.tensor.matmul(pss[g][:],
                                             wt_g[:, wlo:wlo + 128],
                                             xt[:, phi // 64, cc, :],
                                             start=(m == 0), stop=(m == m_hi))
                    if variant not in ("mmint", "intonly"):
                        nc.vector.tensor_copy(stage[:, sb, 0, :], ps0[:])
                        nc.scalar.copy(stage[:, sb, 1, :], ps1[:])
                elif variant != "nomm":
                    for g in range(2):
                        ps = pp.tile([128, 512], f32)
                        for m in range(m_hi + 1):
                            cc = g - c + m + CPADB
                            wlo = loff + 128 * m
                            nc.tensor.matmul(ps[:],
                                             wt_g[:, wlo:wlo + 128],
                                             xt[:, phi // 64, cc, :],
                                             start=(m == 0), stop=(m == m_hi))
                        if variant == "mmonly":
                            continue
                        # evac split: DVE busy with upcasts; ACT takes g=1
                        if g == 0:
                            nc.vector.tensor_copy(stage[:, sb, g, :], ps[:])
                        else:
                            nc.scalar.copy(stage[:, sb, g, :], ps[:])
                else:
                    nc.vector.tensor_copy(stage[:, sb, 0, 0:1], xt[:, 0, 0, 0:1])
                if sb == OBATCH - 1 and variant not in ("noout", "mmonly", "mmevac", "mm32",
                                                        "mmn256", "mmnoacc", "mmint", "intonly"):
                    blk = ((idx + 1 - OBATCH) % NS) // OBATCH
                    nc.scalar.dma_start(
                        out_v[blk],
                        stage[:].rearrange("j o g (k b) -> j o g k b",
                                           k=8, b=BC))
    nc.compile()
    return nc


CPADB = 5
OBATCH = 8   # scales per output DMA batch (64 % OBATCH == 0)
INTERLEAVE = False  # interleave g-chains (alternate PSUM banks per MM)


def _cast_w(Wall):
    import ml_dtypes
    if WDTYPE == "bf16":
        return Wall.astype(ml_dtypes.bfloat16)
    return Wall.astype(np.float16)


def _cast_x(xt):
    import ml_dtypes
    if WDTYPE == "bf16":
        return xt.astype(ml_dtypes.bfloat16)
    return xt


def _shard_x_b(x):
    """x -> per-core [128, 2, 11, 512] where [p, f, cc, k*64+b] =
    x_pad[b, 128*(2k + cc - CPADB) + 64f + p]."""
    xs_all = np.asarray(x, np.float32).reshape(NCORES * BC, SIG)
    shards = []
    for cidx in range(NCORES):
        lin = np.zeros(((NCHUNK + 1) * 128, BC), np.float32)
        lin[LPAD * 128:LPAD * 128 + SIG, :] = xs_all[cidx * BC:(cidx + 1) * BC].T
        xt = np.zeros((128, 2, 11, 512), np.float32)
        for f in range(2):
            for cc in range(11):
                for k in range(8):
                    lo = 128 * (2 * k + cc - CPADB + LPAD) + 64 * f
                    if 0 <= lo and lo + 128 <= lin.shape[0]:
                        xt[:, f, cc, k * BC:(k + 1) * BC] = lin[lo:lo + 128]
        shards.append(np.ascontiguousarray(_cast_x(xt)))
    return shards


def _gather_b(per_core_outs):
    """[NS/OB, 128, OB, 2, 8, BC] per core -> full [B, NS, SIG].
    s = OBATCH*blk + o, n = 256*k + 128*g + j."""
    outs = []
    for o in per_core_outs:
        # o[blk, j, o, g, k, b] -> [b, blk, o, k, g, j] -> reshape [b, s, n]
        t = np.ascontiguousarray(
            np.asarray(o).astype(np.float32).transpose(5, 0, 2, 4, 3, 1))
        outs.append(t.reshape(BC, NS, SIG))
    return np.concatenate(outs, axis=0)


def _shard_x(x):
    """x [512, 1, 2048] -> list of per-core [128, NCHUNK, NH*BC] transposed
    zero-padded signal arrays. Free layout (c, h*BC+b) holds chunk c+8*h,
    so both packed n-halves sit contiguously for the matmul stationary."""
    xs_all = np.asarray(x, np.float32).reshape(NCORES * BC, SIG)
    shards = []
    for cidx in range(NCORES):
        lin = np.zeros((NCHUNK * 128, BC), np.float32)
        lin[LPAD * 128:LPAD * 128 + SIG, :] = xs_all[cidx * BC:(cidx + 1) * BC].T
        ch = lin.reshape(NCHUNK, 128, BC)
        xt = np.zeros((128, NCHUNK, NH * BC), np.float32)
        for h in range(NH):
            n = NCHUNK - 8 * h
            xt[:, :n, h * BC:(h + 1) * BC] = ch[8 * h:].transpose(1, 0, 2)
        shards.append(np.ascontiguousarray(xt))
    return shards


def _get_program(kernels, scales_sqrt, trim_idx):
    key = "prog"
    if key not in _CACHE:
        Wall, specs = _build_filters(kernels, scales_sqrt, trim_idx)
        nc = _build_nc(Wall.shape[1], specs)
        _CACHE[key] = (nc, Wall, specs)
    return _CACHE[key]


TRACE = False  # set True (e.g. from test.py) to capture a neuron profile
PLAN = "b"     # "a": batch-major PSUM; "b": j-major PSUM + host transpose


WDTYPE = "bf16"  # "bf16" | "f32r_up" ("f32r_wf16" rejected by walrus)
# bf16 wins: f32r stationaries self-load serially inside each matmul
# (~267ns/MM); bf16 LDWEIGHTS pipelines through the PE reorder window.


def _get_program_b(kernels, scales_sqrt, trim_idx):
    key = "prog_b" + WDTYPE
    if key not in _CACHE:
        Wall, specs = _build_filters_b(kernels, scales_sqrt, trim_idx)
        nc = _build_nc_b(Wall.shape[1], specs, wdtype=WDTYPE)
        _CACHE[key] = (nc, _cast_w(Wall), specs)
    return _CACHE[key]


def kernel(x, kernels, scales_sqrt, trim_idx):
    from concourse.bass_utils import run_bass_kernel_spmd

    if PLAN == "b":
        nc, Wall, _specs = _get_program_b(kernels, scales_sqrt, trim_idx)
        shards = _shard_x_b(x)
        in_maps = [{"xt": sh, "w": Wall} for sh in shards]
        res = run_bass_kernel_spmd(nc, in_maps, list(range(NCORES)), trace=TRACE)
        _CACHE["last_results"] = res
        out = _gather_b([res.results[i]["out"] for i in range(NCORES)])
        return np.ascontiguousarray(out.astype(np.float32))

    nc, Wall, _specs = _get_program(kernels, scales_sqrt, trim_idx)
    shards = _shard_x(x)
    in_maps = [{"xt": sh, "w": Wall} for sh in shards]
    res = run_bass_kernel_spmd(nc, in_maps, list(range(NCORES)), trace=TRACE)
    _CACHE["last_results"] = res
    out = np.concatenate([res.results[i]["out"] for i in range(NCORES)], axis=0)
    return np.ascontiguousarray(out.astype(np.float32))

